# revision 1
# baseline (speedup 1.0000x reference)
"""Trainium2 Bass kernel for a dense transformer block (B=2, T=2048, C=1024, H=16).

v3 over v2:
  - QKV bias/mean terms folded INTO the QKV PSUM accumulation
    (ps = W.T@x + colsum x (-mu) + bias x sd, out = ps * rstd) -- one DVE
    fixup op per j and one PSUM group, eliminating the ps2 bank pressure.
  - PSUM pools scoped per phase (P1 / attention / post) so each phase gets
    8KB of rotation on top of the 8KB persistent mm/av rings.
  - Post-attention pipeline split by batch half (proj -> resid -> LN2 ->
    fc per 256-token half) so the second AllToAll hides under the first
    half's compute.
  - wfc streamed as 1MB DMAs (4 f-blocks each) for large descriptors.

Sharding: 8-way tensor parallel over heads for QKV+attention (each core
owns 2 heads over all 4096 tokens); a per-batch AllToAll switches to token
parallelism: core i owns tokens b0[256i:+256] + b1[256i:+256] for
attn-proj, LN2 and the MLP.
"""

import math
import sys
from contextlib import ExitStack

import numpy as np

for _p in ("/opt/trn_rl_repo",):
    if _p not in sys.path:
        sys.path.insert(0, _p)

import concourse.bacc as bacc
import concourse.mybir as mybir
import concourse.tile as tile
from concourse.bass_utils import run_bass_kernel_spmd

F32 = mybir.dt.float32
F32R = mybir.dt.float32r
BF16 = mybir.dt.bfloat16
NPBF16 = mybir.dt.np(mybir.dt.bfloat16)

B, T, C = 2, 2048, 1024
H, HD = 16, 64
TT = B * T              # 4096 flat tokens (b0: 0..2047, b1: 2048..4095)
NCORES = 8
CHUNK = 512             # tokens per core for the MLP part (256 per batch)
NC_BLK = C // 128       # 8 feature blocks
NF_BLK = 4 * C // 128   # 32 mlp-hidden blocks
EPS = 1e-5


def build_program(repeat=1, phases=99):
    nc = bacc.Bacc("TRN2", target_bir_lowering=False, debug=False,
                   num_devices=NCORES)

    # ---- I/O (big tensors bf16, per-partition-contiguous layouts) ----
    xT = nc.dram_tensor("xT", [128, NC_BLK, TT], BF16, kind="ExternalInput")
    xc_in = nc.dram_tensor("xc_in", [128, NC_BLK, CHUNK], BF16,
                           kind="ExternalInput")
    wqkv = nc.dram_tensor("wqkv", [128, NC_BLK, 3, 128], BF16,
                          kind="ExternalInput")
    cbqkv = nc.dram_tensor("cbqkv", [1, 6, 128], BF16, kind="ExternalInput")
    wproj = nc.dram_tensor("wproj", [128, NC_BLK, NC_BLK, 128], BF16,
                           kind="ExternalInput")
    bproj = nc.dram_tensor("bproj", [128, NC_BLK], F32, kind="ExternalInput")
    wfc = nc.dram_tensor("wfc", [128, NF_BLK // 4, 4, NC_BLK, 128], BF16,
                         kind="ExternalInput")
    bfc = nc.dram_tensor("bfc", [128, NF_BLK], F32, kind="ExternalInput")
    wmlp = nc.dram_tensor("wmlp", [NC_BLK, 128, NF_BLK, 128], BF16,
                          kind="ExternalInput")
    bmlp = nc.dram_tensor("bmlp", [128, NC_BLK], F32, kind="ExternalInput")
    ones_in = nc.dram_tensor("ones_in", [128, 512], BF16,
                             kind="ExternalInput")
    cmask = nc.dram_tensor("cmask", [128, 4, 512], BF16, kind="ExternalInput")
    identin = nc.dram_tensor("identin", [128, 128], BF16,
                             kind="ExternalInput")
    epsin = nc.dram_tensor("epsin", [128, 1], F32, kind="ExternalInput")
    yout = nc.dram_tensor("yout", [128, NC_BLK, CHUNK], F32,
                          kind="ExternalOutput")

    AFT = mybir.ActivationFunctionType
    ALU = mybir.AluOpType

    with tile.TileContext(nc) as tc, ExitStack() as top:
        psum = top.enter_context(tc.tile_pool(name="psum", bufs=1,
                                              space="PSUM"))
        consts = top.enter_context(tc.tile_pool(name="consts", bufs=1))
        wres = top.enter_context(tc.tile_pool(name="wres", bufs=1))
        rows_pool = top.enter_context(tc.tile_pool(name="rows", bufs=4))
        bcast_pool = top.enter_context(tc.tile_pool(name="bcast", bufs=3))
        work = top.enter_context(tc.tile_pool(name="work", bufs=4))
        dram = top.enter_context(tc.tile_pool(name="dram", bufs=1,
                                              space="DRAM"))

        # ---- constants ----
        ident = consts.tile([128, 128], BF16)
        nc.sync.dma_start(out=ident, in_=identin[:, :])
        ones_bf = consts.tile([128, 512], BF16)
        nc.sync.dma_start(out=ones_bf, in_=ones_in[:, :])
        ones_col = ones_bf[:, 0:1]
        eps_col = consts.tile([128, 1], F32)
        nc.sync.dma_start(out=eps_col, in_=epsin[:, :])
        masks = consts.tile([128, 2, 1024], BF16)  # two band pair-masks
        nc.sync.dma_start(out=masks,
                          in_=cmask.ap().rearrange("p (a b) t -> p a (b t)",
                                                   a=2))
        sb_cbq = consts.tile([1, 6, 128], BF16)
        nc.sync.dma_start(out=sb_cbq, in_=cbqkv[:, :, :])
        sb_bproj = consts.tile([128, NC_BLK], F32)
        nc.sync.dma_start(out=sb_bproj, in_=bproj[:, :])
        sb_bfc = consts.tile([128, NF_BLK], F32)
        nc.sync.dma_start(out=sb_bfc, in_=bfc[:, :])
        sb_bmlp = consts.tile([128, NC_BLK], F32)
        nc.sync.dma_start(out=sb_bmlp, in_=bmlp[:, :])

        # ---- resident weights (loaded once per program) ----
        wq_all = wres.tile([128, NC_BLK, 3, 128], BF16, name="wq_all")
        nc.sync.dma_start(out=wq_all, in_=wqkv[:, :, :, :])
        wp_all = wres.tile([128, NC_BLK, NC_BLK, 128], BF16, name="wp_all")
        nc.sync.dma_start(out=wp_all, in_=wproj[:, :, :, :])

        def bcast_row(row_ap, nparts, dtype, tag="bc", width=512):
            """Broadcast a [1, width] SBUF row to [nparts, width] via a K=1
            PE outer product with a ones row, evacuated to SBUF by DVE."""
            ps = psum.tile([128, 512], F32, tag="mm", bufs=2, name="ps_bc")
            nc.tensor.matmul(ps[0:nparts, 0:width], ones_bf[0:1, 0:nparts],
                             row_ap, start=True, stop=True)
            out = bcast_pool.tile([nparts, width], dtype, tag=tag,
                                  name="bc_row")
            with nc.allow_low_precision(reason="broadcast copy"):
                nc.vector.tensor_copy(out, ps[0:nparts, 0:width])
            return out

        def ln_stats_rows(srcs, sqs, width=512):
            """srcs/sqs: NC_BLK [128, width] bf16 APs (feature blocks of one
            token chunk and their elementwise squares). Returns bf16 rows
            (rstd, nmu=-mu, sd), each [1, width]."""
            ps_s = psum.tile([65, 512], F32, tag="av", bufs=2, name="ps_s")
            ps_q = psum.tile([65, 512], F32, tag="av", bufs=2, name="ps_q")
            for c in range(NC_BLK):
                nc.tensor.matmul(ps_s[0:1, 0:width], ones_col, srcs[c],
                                 start=(c == 0), stop=(c == NC_BLK - 1))
            for c in range(NC_BLK):
                nc.tensor.matmul(ps_q[0:1, 0:width], ones_col, sqs[c],
                                 start=(c == 0), stop=(c == NC_BLK - 1))
            mu = rows_pool.tile([1, width], F32, tag="r")
            nc.vector.tensor_scalar_mul(mu, ps_s[0:1, 0:width], 1.0 / C)
            ex2 = rows_pool.tile([1, width], F32, tag="r")
            nc.vector.tensor_scalar_mul(ex2, ps_q[0:1, 0:width], 1.0 / C)
            var = rows_pool.tile([1, width], F32, tag="r")
            musq = rows_pool.tile([1, width], F32, tag="r")
            nc.vector.tensor_mul(musq, mu, mu)
            nc.vector.tensor_sub(var, ex2, musq)
            sd = rows_pool.tile([1, width], F32, tag="r")
            nc.scalar.activation(out=sd, in_=var, func=AFT.Sqrt,
                                 bias=eps_col[0:1, 0:1])
            rstd = rows_pool.tile([1, width], BF16, tag="rb", bufs=2)
            nmu = rows_pool.tile([1, width], BF16, tag="rb", bufs=2)
            sdb = rows_pool.tile([1, width], BF16, tag="rb", bufs=2)
            with nc.allow_low_precision(reason="bf16 rows"):
                nc.vector.reciprocal(rstd, sd)
                nc.vector.tensor_scalar_mul(nmu, mu, -1.0)
                nc.vector.tensor_copy(sdb, sd)
            return rstd, nmu, sdb

        def emit_body(rep):
            with ExitStack() as body_scope:
                body = body_scope.enter_context(
                    tc.tile_pool(name=f"body{rep}", bufs=1))
                xct = body.tile([128, NC_BLK, CHUNK], BF16, name="xct")
                residT = body.tile([128, NC_BLK, CHUNK], BF16, name="residT")
                ln2T = body.tile([128, NC_BLK, CHUNK], BF16, name="ln2T")
                hT = body.tile([128, NF_BLK, CHUNK], BF16, name="hT")
                yfull = body.tile([128, NC_BLK, CHUNK], BF16, name="yfull")

                # residual-chunk prefetch (per-core input; independent)
                nc.sync.dma_start(out=xct, in_=xc_in[:, :, :])

                with ExitStack() as attn_scope:
                    attn_pool = attn_scope.enter_context(
                        tc.tile_pool(name=f"attn{rep}", bufs=1))
                    qkT = attn_pool.tile([128, 2, TT], BF16, name="qkT")
                    vones = attn_pool.tile([128, 2, TT // 128, 65], BF16,
                                           name="vones")
                    yT = attn_pool.tile([128, B, T], BF16, name="yT")
                    nc.vector.tensor_copy(
                        vones[:, :, :, 64:65].rearrange(
                            "p a b k -> p (a b k)"),
                        ones_bf[:, 0:64])

                    # ========== Phase 1: LN1 stats + QKV ==========
                    with ExitStack() as p1_scope:
                        xc_pool = p1_scope.enter_context(
                            tc.tile_pool(name=f"xcp{rep}", bufs=2))
                        psq = p1_scope.enter_context(
                            tc.tile_pool(name=f"psq{rep}", bufs=1,
                                         space="PSUM"))
                        for qi in range(8):
                            csl = slice(512 * qi, 512 * (qi + 1))
                            xTc = xc_pool.tile([128, NC_BLK, 512], BF16,
                                               tag="xTc", name="xTc")
                            nc.sync.dma_start(out=xTc, in_=xT[:, :, csl])
                            srcs = [xTc[:, c, :] for c in range(NC_BLK)]
                            sq = xc_pool.tile([128, NC_BLK, 512], BF16,
                                              tag="sq", name="sq")
                            with nc.allow_low_precision(reason="bf16 sq"):
                                for c in range(NC_BLK):
                                    nc.vector.tensor_mul(sq[:, c, :],
                                                         srcs[c], srcs[c])
                            rstd, nmu, sdb = ln_stats_rows(
                                srcs, [sq[:, c, :] for c in range(NC_BLK)])
                            rstd_b = bcast_row(rstd[0:1, :], 128, BF16)
                            ps_j = []
                            for j in range(3):
                                ps = psq.tile([128, 512], F32, tag="q5",
                                              bufs=4, name="ps_qkv")
                                for c in range(NC_BLK):
                                    nc.tensor.matmul(ps, wq_all[:, c, j, :],
                                                     srcs[c],
                                                     start=(c == 0),
                                                     stop=False)
                                nc.tensor.matmul(ps, sb_cbq[:, j, :],
                                                 nmu[:, :],
                                                 start=False, stop=False)
                                nc.tensor.matmul(ps, sb_cbq[:, 3 + j, :],
                                                 sdb[:, :],
                                                 start=False, stop=True)
                                ps_j.append(ps)
                            for j in range(3):
                                with nc.allow_low_precision(reason="bf16"):
                                    if j < 2:
                                        nc.vector.tensor_mul(
                                            qkT[:, j, csl], ps_j[j], rstd_b)
                                    else:
                                        vch = work.tile([128, 512], BF16,
                                                        tag="vch", bufs=2,
                                                        name="vch")
                                        nc.vector.tensor_mul(vch, ps_j[j],
                                                             rstd_b)
                                        for kb in range(4):
                                            ps_t = psum.tile(
                                                [128, 128], BF16,
                                                tag="mm", bufs=2,
                                                name="ps_tr")
                                            nc.tensor.transpose(
                                                ps_t,
                                                vch[:, 128 * kb:
                                                    128 * (kb + 1)],
                                                ident)
                                            gb = 4 * qi + kb
                                            for hh in range(2):
                                                nc.vector.tensor_copy(
                                                    vones[:, hh, gb, 0:64],
                                                    ps_t[:, 64 * hh:
                                                         64 * hh + 64])
                    if phases <= 1:
                        return

                    # ====== Phase 2: causal attention, b-major ======
                    a2a_in = [dram.tile([NCORES, 128, 256], BF16,
                                        name=f"a2a_in{b}") for b in range(B)]
                    a2a_out = [dram.tile([NCORES, 128, 256], BF16,
                                         name=f"a2a_out{b}")
                               for b in range(B)]
                    inv_sqrt_hd = 1.0 / math.sqrt(HD)
                    with ExitStack() as pa_scope:
                        psa = pa_scope.enter_context(
                            tc.tile_pool(name=f"psa{rep}", bufs=1,
                                         space="PSUM"))
                        for b in range(B):
                            for hh in range(2):
                                hsl = slice(64 * hh, 64 * hh + 64)
                                for ql in range(4):
                                    npair = 2 * ql + 2
                                    q_sl = slice(T * b + 512 * ql,
                                                 T * b + 512 * (ql + 1))
                                    ps_y = psum.tile([65, 512], F32,
                                                     tag="av", bufs=2,
                                                     name="ps_y")
                                    for pp in range(npair):
                                        pair = psa.tile([128, 2, 512], F32,
                                                        tag="qk", bufs=2,
                                                        name="ps_qk")
                                        for half in range(2):
                                            k = 2 * pp + half
                                            k_sl = slice(
                                                T * b + 128 * k,
                                                T * b + 128 * (k + 1))
                                            nc.tensor.matmul(
                                                pair[:, half, :],
                                                qkT[hsl, 1, k_sl],
                                                qkT[hsl, 0, q_sl],
                                                start=True, stop=True)
                                        est = work.tile([128, 2, 512], BF16,
                                                        tag="est", bufs=3,
                                                        name="est")
                                        nc.scalar.activation(
                                            out=est.rearrange(
                                                "p a t -> p (a t)"),
                                            in_=pair.rearrange(
                                                "p a t -> p (a t)"),
                                            func=AFT.Exp, scale=inv_sqrt_hd)
                                        m2 = pp - 2 * ql
                                        if m2 >= 0:
                                            with nc.allow_low_precision(
                                                    reason="bf16 mask"):
                                                nc.vector.tensor_mul(
                                                    est.rearrange(
                                                        "p a t -> p (a t)"),
                                                    est.rearrange(
                                                        "p a t -> p (a t)"),
                                                    masks[:, m2, :])
                                        for half in range(2):
                                            k = 2 * pp + half
                                            nc.tensor.matmul(
                                                ps_y[0:65, :],
                                                vones[:, hh,
                                                      (T * b) // 128 + k, :],
                                                est[:, half, :],
                                                start=(pp == 0 and
                                                       half == 0),
                                                stop=(pp == npair - 1
                                                      and half == 1))
                                    srow = rows_pool.tile([1, 512], BF16,
                                                          tag="sr", bufs=2,
                                                          name="srow")
                                    with nc.allow_low_precision(
                                            reason="bf16"):
                                        nc.vector.reciprocal(
                                            srow[0:1, :], ps_y[64:65, :])
                                    rb = bcast_row(srow[0:1, :], 64, F32,
                                                   tag="rb")
                                    with nc.allow_low_precision(
                                            reason="bf16 y"):
                                        nc.vector.tensor_mul(
                                            yT[64 * hh:64 * hh + 64, b,
                                               512 * ql:512 * (ql + 1)],
                                            ps_y[0:64, :], rb[0:64, :])
                            if phases <= 3:
                                continue
                            # ---- per-batch AllToAll ----
                            nc.sync.dma_start(
                                out=a2a_in[b].rearrange("j p t -> p j t"),
                                in_=yT[:, b, :].rearrange(
                                    "p (j t) -> p j t", j=NCORES))
                            nc.gpsimd.collective_compute(
                                "AllToAll", ALU.bypass,
                                replica_groups=[list(range(NCORES))],
                                ins=[a2a_in[b][:]], outs=[a2a_out[b][:]],
                            )
                            # collective-dependent load on the ACT queue
                            nc.scalar.dma_start(
                                out=yfull[:, :, 256 * b:256 * (b + 1)],
                                in_=a2a_out[b].rearrange("j p t -> p j t"))
                if phases <= 4:
                    return

                # ==== Phases 4-6 split by batch half: proj+resid,
                # ==== LN2, fc -- half 0 runs while A2A(b1) flies
                with ExitStack() as pp_scope:
                    p6_pool = pp_scope.enter_context(
                        tc.tile_pool(name=f"p6_{rep}", bufs=1))
                    psp = pp_scope.enter_context(
                        tc.tile_pool(name=f"psp{rep}", bufs=1,
                                     space="PSUM"))
                    for bh in range(2):
                        tsl = slice(256 * bh, 256 * (bh + 1))
                        for co in range(NC_BLK):
                            ps = psp.tile([128, 512], F32, tag="pf",
                                          bufs=4, name="ps_pj")
                            for ci in range(NC_BLK):
                                nc.tensor.matmul(
                                    ps[:, 0:256],
                                    wp_all[:, ci, co, :],
                                    yfull[:, ci, tsl],
                                    start=(ci == 0),
                                    stop=(ci == NC_BLK - 1))
                            with nc.allow_low_precision(
                                    reason="bf16 resid"):
                                nc.vector.scalar_tensor_tensor(
                                    out=residT[:, co, tsl],
                                    in0=ps[:, 0:256],
                                    scalar=sb_bproj[:, co:co + 1],
                                    in1=xct[:, co, tsl],
                                    op0=ALU.add, op1=ALU.add)
                        if phases <= 5:
                            continue
                        # ---- LN2 for this half ----
                        sq2 = work.tile([128, NC_BLK, 256], BF16,
                                        tag="sq2", bufs=2, name="sq2")
                        with nc.allow_low_precision(reason="bf16 sq"):
                            for c in range(NC_BLK):
                                nc.vector.tensor_mul(
                                    sq2[:, c, :], residT[:, c, tsl],
                                    residT[:, c, tsl])
                        rstd2, nmu2, _ = ln_stats_rows(
                            [residT[:, c, tsl] for c in range(NC_BLK)],
                            [sq2[:, c, :] for c in range(NC_BLK)],
                            width=256)
                        nmr2 = rows_pool.tile([1, 256], BF16, tag="sr",
                                              bufs=2, name="nmr2")
                        with nc.allow_low_precision(reason="bf16"):
                            nc.vector.tensor_mul(nmr2, nmu2, rstd2)
                        rstd2_b = bcast_row(rstd2[0:1, :], 128, BF16,
                                            width=256)
                        nmr2_b = bcast_row(nmr2[0:1, :], 128, BF16,
                                           tag="bc2", width=256)
                        with nc.allow_low_precision(reason="bf16 ln2"):
                            for c in range(NC_BLK):
                                t5 = work.tile([128, 256], BF16,
                                               tag="wk", name="t5")
                                nc.vector.tensor_mul(
                                    t5, residT[:, c, tsl], rstd2_b)
                                nc.vector.tensor_add(
                                    ln2T[:, c, tsl], t5, nmr2_b)
                        if phases <= 6:
                            continue
                        # ---- MLP fc + gelu for this half ----
                        for fg in range(NF_BLK // 4):
                            wf_t = p6_pool.tile([128, 4, NC_BLK, 128],
                                                BF16, tag="wf", bufs=3,
                                                name="wf")
                            nc.sync.dma_start(out=wf_t,
                                              in_=wfc[:, fg, :, :, :])
                            for fo in range(4):
                                f = 4 * fg + fo
                                ps = psp.tile([128, 512], F32,
                                              tag="pf", bufs=4,
                                              name="ps_fc")
                                for c in range(NC_BLK):
                                    nc.tensor.matmul(
                                        ps[:, 0:256],
                                        wf_t[:, fo, c, :],
                                        ln2T[:, c, tsl],
                                        start=(c == 0),
                                        stop=(c == NC_BLK - 1))
                                nc.scalar.activation(
                                    out=hT[:, f, tsl], in_=ps[:, 0:256],
                                    func=AFT.Gelu,
                                    bias=sb_bfc[:, f:f + 1])
                    if phases <= 7:
                        return

                    # ---- MLP proj + residual + out (full width) ----
                    for co in range(NC_BLK):
                        wm = p6_pool.tile([128, NF_BLK, 128], BF16,
                                          tag="wm", bufs=3, name="wm")
                        nc.gpsimd.dma_start(out=wm, in_=wmlp[co, :, :, :])
                        ps = psp.tile([128, 512], F32, tag="pf",
                                      bufs=4, name="ps_mp")
                        for f in range(NF_BLK):
                            nc.tensor.matmul(ps, wm[:, f, :],
                                             hT[:, f, :],
                                             start=(f == 0),
                                             stop=(f == NF_BLK - 1))
                        yo = work.tile([128, 512], F32, tag="yo",
                                       bufs=2, name="yo")
                        nc.vector.scalar_tensor_tensor(
                            out=yo, in0=ps,
                            scalar=sb_bmlp[:, co:co + 1],
                            in1=residT[:, co, :],
                            op0=ALU.add, op1=ALU.add)
                        nc.sync.dma_start(out=yout[:, co, :], in_=yo)

        for _rep in range(repeat):
            emit_body(_rep)

    nc.compile()
    return nc


_NC_CACHE = {}


def _get_program(repeat=1):
    if repeat not in _NC_CACHE:
        _NC_CACHE[repeat] = build_program(repeat)
    return _NC_CACHE[repeat]


def prepare_inputs(x, ln1_g, ln1_b, w_attn, b_attn, w_attn_proj, b_attn_proj,
                   ln2_g, ln2_b, w_fc, b_fc, w_mlp_proj, b_mlp_proj):
    """Host-side fold/slice/block. Returns in_maps (list of 8 dicts)."""
    f = np.float32
    bf = NPBF16
    x = np.asarray(x, f)
    # fold LN1 gain into w_attn, LN2 gain into w_fc (exact: reference
    # applies g/b after normalization; W'.T @ (g*xn + b) = (g*W)'.T @ xn
    # + (b @ W))
    w_attn_e = (np.asarray(ln1_g, f)[:, None] * np.asarray(w_attn, f))
    b_attn_e = np.asarray(ln1_b, f) @ np.asarray(w_attn, f) + \
        np.asarray(b_attn, f)
    w_fc_e = (np.asarray(ln2_g, f)[:, None] * np.asarray(w_fc, f))
    b_fc_e = np.asarray(ln2_b, f) @ np.asarray(w_fc, f) + np.asarray(b_fc, f)
    colsum = w_attn_e.sum(axis=0, dtype=np.float64).astype(f)

    xT = np.concatenate([x[0].T, x[1].T], axis=1)          # [C, 4096]
    xT_blk = np.ascontiguousarray(
        xT.reshape(NC_BLK, 128, TT).transpose(1, 0, 2)).astype(bf)

    # wproj: [C, C] -> [p(ci-row), ci, co, k]
    wp = np.ascontiguousarray(
        np.asarray(w_attn_proj, f).reshape(NC_BLK, 128, NC_BLK, 128)
        .transpose(1, 0, 2, 3)).astype(bf)
    # wfc: [C, 4C] -> [p(c-row), fg, fo, c, k] (1MB per-fg DMA slices)
    wf = np.ascontiguousarray(
        w_fc_e.reshape(NC_BLK, 128, NF_BLK // 4, 4, 128)
        .transpose(1, 2, 3, 0, 4)).astype(bf)
    # wmlp: [4C, C] -> [co, p(f-row), f, k]
    wm = np.ascontiguousarray(
        np.asarray(w_mlp_proj, f).reshape(NF_BLK, 128, NC_BLK, 128)
        .transpose(2, 1, 0, 3)).astype(bf)

    def rows_t(v, nb):
        return np.ascontiguousarray(np.asarray(v, f).reshape(nb, 128).T)

    bproj_t = rows_t(b_attn_proj, NC_BLK)
    bfc_t = rows_t(b_fc_e, NF_BLK)
    bmlp_t = rows_t(b_mlp_proj, NC_BLK)

    ones_arr = np.ones((128, 512), bf)
    ident_arr = np.eye(128).astype(bf)
    eps_arr = np.full((128, 1), EPS, f)
    # causal mask m: [p, c] valid (1.0) iff p + 128*m <= c
    p_idx = np.arange(128)[:, None]
    c_idx = np.arange(512)[None, :]
    cmask_arr = np.ascontiguousarray(np.stack(
        [(p_idx + 128 * m <= c_idx).astype(f) for m in range(4)],
        axis=1)).astype(bf)

    in_maps = []
    for i in range(NCORES):
        qcols = slice(128 * i, 128 * (i + 1))
        kcols = slice(C + 128 * i, C + 128 * (i + 1))
        vcols = slice(2 * C + 128 * i, 2 * C + 128 * (i + 1))
        wq = np.empty((128, NC_BLK, 3, 128), f)
        for c in range(NC_BLK):
            rsl = slice(128 * c, 128 * (c + 1))
            wq[:, c, 0, :] = w_attn_e[rsl, qcols]
            wq[:, c, 1, :] = w_attn_e[rsl, kcols]
            wq[:, c, 2, :] = w_attn_e[rsl, vcols]
        cb = np.empty((1, 6, 128), f)
        for j, sl in enumerate((qcols, kcols, vcols)):
            cb[0, j, :] = colsum[sl]
            cb[0, 3 + j, :] = b_attn_e[sl]
        # per-core residual chunk: b0[256i:256(i+1)] ++ b1[256i:256(i+1)]
        xc = np.concatenate(
            [xT_blk[:, :, 256 * i:256 * (i + 1)],
             xT_blk[:, :, 2048 + 256 * i:2048 + 256 * (i + 1)]], axis=2)
        in_maps.append({
            "ones_in": ones_arr,
            "cmask": cmask_arr,
            "identin": ident_arr,
            "epsin": eps_arr,
            "xT": xT_blk,
            "xc_in": np.ascontiguousarray(xc),
            "wqkv": wq.astype(bf),
            "cbqkv": cb.astype(bf),
            "wproj": wp,
            "bproj": bproj_t,
            "wfc": wf,
            "bfc": bfc_t,
            "wmlp": wm,
            "bmlp": bmlp_t,
        })
    return in_maps


def assemble_output(results):
    out = np.empty((B, T, C), np.float32)
    for i in range(NCORES):
        yo = results[i]["yout"]                      # [128, 8, 512]
        y = yo.transpose(1, 0, 2).reshape(C, CHUNK)  # [feature, 512]
        out[0, 256 * i:256 * (i + 1), :] = y[:, 0:256].T
        out[1, 256 * i:256 * (i + 1), :] = y[:, 256:512].T
    return out


def kernel(**inputs):
    nc = _get_program()
    in_maps = prepare_inputs(**inputs)
    res = run_bass_kernel_spmd(nc, in_maps, list(range(NCORES)))
    return assemble_output(res.results)


if __name__ == "__main__":
    import reference
    inputs = {k: np.asarray(v) for k, v in reference.setup_inputs().items()}
    expected = np.asarray(reference.reference(**inputs))
    actual = kernel(**inputs)
    err = np.abs(actual - expected).max() / (np.abs(expected).max() + 1e-30)
    print("Relative error:", err)



# revision 12
# speedup vs baseline: 1.0835x; 1.0835x over previous
"""Trainium2 Bass kernel for a dense transformer block (B=2, T=2048, C=1024, H=16).

v4 over v3 (all scheduling, no math changes):
  - Attention restructured as TWO interleaved head-streams per batch so
    each pair's softmax-exp (ScalarE) overlaps the other stream's QK/AV
    matmuls (v3 serialized QK->exp->AV per pair: ~2.7us/pair).
  - A2A input DMA + collective + consume DMA all on the gpsimd queue
    (v3 had the consume on the ACT queue, stalling batch-1 exps behind
    the batch-0 collective).
  - wfc/wmlp streamed on the SP HWDGE queue with all dma_starts emitted
    right after P1, so 3+3 MB prefetch during attention while HBM is
    otherwise idle (v3 issued wmlp late on the gpsimd SWDGE queue).
  - fc is fg-outer, full 512-token width: wfc read once (v3 streamed it
    per batch-half = 2x traffic) and N=512 matmuls/gelu.

Sharding: 8-way tensor parallel over heads for QKV+attention (each core
owns 2 heads over all 4096 tokens); a per-batch AllToAll switches to token
parallelism: core i owns tokens b0[256i:+256] + b1[256i:+256] for
attn-proj, LN2 and the MLP.
"""

import math
import sys
from contextlib import ExitStack

import numpy as np

for _p in ("/opt/trn_rl_repo",):
    if _p not in sys.path:
        sys.path.insert(0, _p)

import concourse.bacc as bacc
import concourse.mybir as mybir
import concourse.tile as tile
from concourse.bass_utils import run_bass_kernel_spmd

F32 = mybir.dt.float32
BF16 = mybir.dt.bfloat16
NPBF16 = mybir.dt.np(mybir.dt.bfloat16)

B, T, C = 2, 2048, 1024
H, HD = 16, 64
TT = B * T              # 4096 flat tokens (b0: 0..2047, b1: 2048..4095)
NCORES = 8
CHUNK = 512             # tokens per core for the MLP part (256 per batch)
NC_BLK = C // 128       # 8 feature blocks
NF_BLK = 4 * C // 128   # 32 mlp-hidden blocks
EPS = 1e-5


def build_program(repeat=1, phases=99):
    nc = bacc.Bacc("TRN2", target_bir_lowering=False, debug=False,
                   num_devices=NCORES)

    # ---- I/O (big tensors bf16, per-partition-contiguous layouts) ----
    xT = nc.dram_tensor("xT", [128, NC_BLK, TT], BF16, kind="ExternalInput")
    xc_in = nc.dram_tensor("xc_in", [128, NC_BLK, CHUNK], BF16,
                           kind="ExternalInput")
    wqkv = nc.dram_tensor("wqkv", [128, NC_BLK, 3, 128], BF16,
                          kind="ExternalInput")
    cbqkv = nc.dram_tensor("cbqkv", [1, 6, 128], BF16, kind="ExternalInput")
    wproj = nc.dram_tensor("wproj", [128, NC_BLK, NC_BLK, 128], BF16,
                           kind="ExternalInput")
    bproj = nc.dram_tensor("bproj", [128, NC_BLK], F32, kind="ExternalInput")
    wfc = nc.dram_tensor("wfc", [128, NF_BLK // 4, 4, NC_BLK, 128], BF16,
                         kind="ExternalInput")
    bfc = nc.dram_tensor("bfc", [128, NF_BLK], F32, kind="ExternalInput")
    wmlp = nc.dram_tensor("wmlp", [NC_BLK, 128, NF_BLK, 128], BF16,
                          kind="ExternalInput")
    bmlp = nc.dram_tensor("bmlp", [128, NC_BLK], F32, kind="ExternalInput")
    ones_in = nc.dram_tensor("ones_in", [128, 512], BF16,
                             kind="ExternalInput")
    cmask = nc.dram_tensor("cmask", [128, 4, 512], BF16, kind="ExternalInput")
    identin = nc.dram_tensor("identin", [128, 128], BF16,
                             kind="ExternalInput")
    epsin = nc.dram_tensor("epsin", [128, 1], F32, kind="ExternalInput")
    yout = nc.dram_tensor("yout", [128, NC_BLK, CHUNK], F32,
                          kind="ExternalOutput")

    AFT = mybir.ActivationFunctionType
    ALU = mybir.AluOpType

    with tile.TileContext(nc) as tc, ExitStack() as top:
        psum = top.enter_context(tc.tile_pool(name="psum", bufs=1,
                                              space="PSUM"))
        consts = top.enter_context(tc.tile_pool(name="consts", bufs=1))
        wres = top.enter_context(tc.tile_pool(name="wres", bufs=1))
        rows_pool = top.enter_context(tc.tile_pool(name="rows", bufs=4))
        bcast_pool = top.enter_context(tc.tile_pool(name="bcast", bufs=2))
        work = top.enter_context(tc.tile_pool(name="work", bufs=4))
        dram = top.enter_context(tc.tile_pool(name="dram", bufs=1,
                                              space="DRAM"))

        # ---- constants ----
        ident = consts.tile([128, 128], BF16)
        nc.sync.dma_start(out=ident, in_=identin[:, :])
        ones_bf = consts.tile([128, 512], BF16)
        nc.sync.dma_start(out=ones_bf, in_=ones_in[:, :])
        ones_col = ones_bf[:, 0:1]
        eps_col = consts.tile([128, 1], F32)
        nc.sync.dma_start(out=eps_col, in_=epsin[:, :])
        masks = consts.tile([128, 2, 1024], BF16)  # two band pair-masks
        nc.sync.dma_start(out=masks,
                          in_=cmask.ap().rearrange("p (a b) t -> p a (b t)",
                                                   a=2))
        sb_cbq = consts.tile([1, 6, 128], BF16)
        nc.sync.dma_start(out=sb_cbq, in_=cbqkv[:, :, :])
        sb_bproj = consts.tile([128, NC_BLK], F32)
        nc.sync.dma_start(out=sb_bproj, in_=bproj[:, :])
        sb_bfc = consts.tile([128, NF_BLK], F32)
        nc.sync.dma_start(out=sb_bfc, in_=bfc[:, :])
        sb_bmlp = consts.tile([128, NC_BLK], F32)
        nc.sync.dma_start(out=sb_bmlp, in_=bmlp[:, :])

        # ---- resident weights (loaded once per program) ----
        wq_all = wres.tile([128, NC_BLK, 3, 128], BF16, name="wq_all")
        nc.sync.dma_start(out=wq_all, in_=wqkv[:, :, :, :])
        wp_all = wres.tile([128, NC_BLK, NC_BLK, 128], BF16, name="wp_all")
        nc.sync.dma_start(out=wp_all, in_=wproj[:, :, :, :])

        def bcast_row(row_ap, nparts, dtype, tag="bc", width=512):
            """Broadcast a [1, width] SBUF row to [nparts, width] via a K=1
            PE outer product with a ones row, evacuated to SBUF by DVE."""
            ps = psum.tile([128, 512], F32, tag="mm", bufs=2, name="ps_bc")
            nc.tensor.matmul(ps[0:nparts, 0:width], ones_bf[0:1, 0:nparts],
                             row_ap, start=True, stop=True)
            out = bcast_pool.tile([nparts, width], dtype, tag=tag,
                                  name="bc_row")
            with nc.allow_low_precision(reason="broadcast copy"):
                nc.vector.tensor_copy(out, ps[0:nparts, 0:width])
            return out

        def finish_stats(ps_s, ps_q, width=512):
            """Turn Σx (ps_s) and Σx² (ps_q) rows into bf16 stat rows:
            rstd [1,w] and nmsd [2,w] (row0 = -mu, row1 = sd)."""
            mu = rows_pool.tile([1, width], F32, tag="r")
            nc.vector.tensor_scalar_mul(mu, ps_s[0:1, 0:width], 1.0 / C)
            ex2 = rows_pool.tile([1, width], F32, tag="r")
            nc.vector.tensor_scalar_mul(ex2, ps_q[0:1, 0:width], 1.0 / C)
            var = rows_pool.tile([1, width], F32, tag="r")
            musq = rows_pool.tile([1, width], F32, tag="r")
            nc.vector.tensor_mul(musq, mu, mu)
            nc.vector.tensor_sub(var, ex2, musq)
            sd = rows_pool.tile([1, width], F32, tag="r")
            nc.scalar.activation(out=sd, in_=var, func=AFT.Sqrt,
                                 bias=eps_col[0:1, 0:1])
            rstd = rows_pool.tile([1, width], BF16, tag="rb", bufs=2)
            nmu = rows_pool.tile([1, width], BF16, tag="rb", bufs=2)
            sdb = rows_pool.tile([1, width], BF16, tag="rb", bufs=2)
            with nc.allow_low_precision(reason="bf16 rows"):
                nc.vector.reciprocal(rstd, sd)
                nc.vector.tensor_scalar_mul(nmu, mu, -1.0)
                nc.vector.tensor_copy(sdb, sd)
            return rstd, nmu, sdb

        def ln_stats_rows(srcs, sqs, width=512):
            """srcs/sqs: NC_BLK [128, width] bf16 APs (feature blocks of one
            token chunk and their elementwise squares)."""
            ps_s = psum.tile([65, 512], F32, tag="av", bufs=2, name="ps_s")
            ps_q = psum.tile([65, 512], F32, tag="av", bufs=2, name="ps_q")
            for c in range(NC_BLK):
                nc.tensor.matmul(ps_s[0:1, 0:width], ones_col, srcs[c],
                                 start=(c == 0), stop=(c == NC_BLK - 1))
            for c in range(NC_BLK):
                nc.tensor.matmul(ps_q[0:1, 0:width], ones_col, sqs[c],
                                 start=(c == 0), stop=(c == NC_BLK - 1))
            return finish_stats(ps_s, ps_q, width)

        def emit_body(rep):
            with ExitStack() as body_scope:
                body = body_scope.enter_context(
                    tc.tile_pool(name=f"body{rep}", bufs=1))
                wstream = body_scope.enter_context(
                    tc.tile_pool(name=f"wstr{rep}", bufs=1))
                xct = body.tile([128, NC_BLK, CHUNK], BF16, name="xct")
                residT = body.tile([128, NC_BLK, CHUNK], BF16, name="residT")
                hT = body.tile([128, NF_BLK, CHUNK], BF16, name="hT")
                yfull = body.tile([128, NC_BLK, CHUNK], BF16, name="yfull")

                with ExitStack() as attn_scope:
                    attn_pool = attn_scope.enter_context(
                        tc.tile_pool(name=f"attn{rep}", bufs=1))
                    qkT = attn_pool.tile([128, 2, TT], BF16, name="qkT")
                    vones = attn_pool.tile([128, 2, TT // 128, 65], BF16,
                                           name="vones")
                    yT = attn_pool.tile([128, B, T], BF16, name="yT")
                    nc.vector.tensor_copy(
                        vones[:, :, :, 64:65].rearrange(
                            "p a b k -> p (a b k)"),
                        ones_bf[:, 0:64])

                    # ========== Phase 1: LN1 stats + QKV ==========
                    with ExitStack() as p1_scope:
                        xc_pool = p1_scope.enter_context(
                            tc.tile_pool(name=f"xcp{rep}", bufs=2))
                        psq = p1_scope.enter_context(
                            tc.tile_pool(name=f"psq{rep}", bufs=1,
                                         space="PSUM"))
                        for qi in range(8):
                            csl = slice(512 * qi, 512 * (qi + 1))
                            xTc = xc_pool.tile([128, NC_BLK, 512], BF16,
                                               tag="xTc", name="xTc")
                            nc.sync.dma_start(out=xTc, in_=xT[:, :, csl])
                            srcs = [xTc[:, c, :] for c in range(NC_BLK)]
                            ps_s = psum.tile([65, 512], F32, tag="av",
                                             bufs=2, name="ps_s")
                            ps_q = psum.tile([65, 512], F32, tag="av",
                                             bufs=2, name="ps_q")
                            for c in range(NC_BLK):
                                nc.tensor.matmul(ps_s[0:1, :], ones_col,
                                                 srcs[c], start=(c == 0),
                                                 stop=(c == NC_BLK - 1))
                            for cc in range(4):
                                sqt = xc_pool.tile([128, 2, 512], BF16,
                                                   tag="sq", bufs=1,
                                                   name="sq")
                                with nc.allow_low_precision(reason="sq"):
                                    for k in range(2):
                                        nc.vector.tensor_mul(
                                            sqt[:, k, :], srcs[2 * cc + k],
                                            srcs[2 * cc + k])
                                for k in range(2):
                                    nc.tensor.matmul(
                                        ps_q[0:1, :], ones_col,
                                        sqt[:, k, :],
                                        start=(cc == 0 and k == 0),
                                        stop=(cc == 3 and k == 1))
                            rstd, nmu, sdb = finish_stats(ps_s, ps_q)
                            rstd_b = bcast_row(rstd[0:1, :], 128, BF16)
                            ps_j = []
                            for j in range(3):
                                ps = psq.tile([128, 512], F32, tag="q5",
                                              bufs=4, name="ps_qkv")
                                for c in range(NC_BLK):
                                    nc.tensor.matmul(ps, wq_all[:, c, j, :],
                                                     srcs[c],
                                                     start=(c == 0),
                                                     stop=False)
                                nc.tensor.matmul(ps, sb_cbq[:, j, :],
                                                 nmu[:, :],
                                                 start=False, stop=False)
                                nc.tensor.matmul(ps, sb_cbq[:, 3 + j, :],
                                                 sdb[:, :],
                                                 start=False, stop=True)
                                ps_j.append(ps)
                            for j in range(3):
                                with nc.allow_low_precision(reason="bf16"):
                                    if j < 2:
                                        nc.vector.tensor_mul(
                                            qkT[:, j, csl], ps_j[j], rstd_b)
                                    else:
                                        vch = work.tile([128, 512], BF16,
                                                        tag="vch", bufs=2,
                                                        name="vch")
                                        nc.vector.tensor_mul(vch, ps_j[j],
                                                             rstd_b)
                                        for kb in range(4):
                                            ps_t = psum.tile(
                                                [128, 128], BF16,
                                                tag="mm", bufs=2,
                                                name="ps_tr")
                                            nc.tensor.transpose(
                                                ps_t,
                                                vch[:, 128 * kb:
                                                    128 * (kb + 1)],
                                                ident)
                                            gb = 4 * qi + kb
                                            for hh in range(2):
                                                nc.vector.tensor_copy(
                                                    vones[:, hh, gb, 0:64],
                                                    ps_t[:, 64 * hh:
                                                         64 * hh + 64])

                    # residual chunk for this core (lands during attention)
                    nc.sync.dma_start(out=xct, in_=xc_in[:, :, :])

                    # weight streams on the SP HWDGE queue. The first 3+3
                    # loads are emitted here so they fire during attention
                    # while HBM is idle; the rest are emitted inside the
                    # fc/mlp loops with bufs-deep lookahead (emission order
                    # = SP queue order, so no head-of-line blocking).
                    wf_tiles = {}
                    wm_tiles = {}

                    def wf_load(fg):
                        t = wstream.tile([128, 4, NC_BLK, 128], BF16,
                                         tag="wf", bufs=2, name=f"wf{fg}")
                        nc.sync.dma_start(out=t, in_=wfc[:, fg, :, :, :])
                        wf_tiles[fg] = t

                    def wm_load(co):
                        t = wstream.tile([128, NF_BLK, 128], BF16,
                                         tag="wm", bufs=2, name=f"wm{co}")
                        nc.sync.dma_start(out=t, in_=wmlp[co, :, :, :])
                        wm_tiles[co] = t

                    for fg in range(2):
                        wf_load(fg)
                    for co in range(2):
                        wm_load(co)
                    if phases <= 1:
                        return

                    # ====== Phase 2: causal attention, two interleaved
                    # ====== head-streams per batch ======
                    a2a_in = [dram.tile([NCORES, 128, 256], BF16,
                                        name=f"a2a_in{b}") for b in range(B)]
                    a2a_out = [dram.tile([NCORES, 128, 256], BF16,
                                         name=f"a2a_out{b}")
                               for b in range(B)]
                    inv_sqrt_hd = 1.0 / math.sqrt(HD)
                    pair_list = [(ql, pp) for ql in range(4)
                                 for pp in range(2 * ql + 2)]
                    with ExitStack() as pa_scope:
                        psa = pa_scope.enter_context(
                            tc.tile_pool(name=f"psa{rep}", bufs=1,
                                         space="PSUM"))
                        for b in range(B):
                            psy = [None, None]
                            ests = [None, None]

                            def qk_exp(hh, ql, pp):
                                hsl = slice(64 * hh, 64 * hh + 64)
                                q_sl = slice(T * b + 512 * ql,
                                             T * b + 512 * (ql + 1))
                                pair = psa.tile([128, 2, 512], F32,
                                                tag="qk", bufs=2,
                                                name="ps_qk")
                                for half in range(2):
                                    k = 2 * pp + half
                                    k_sl = slice(T * b + 128 * k,
                                                 T * b + 128 * (k + 1))
                                    nc.tensor.matmul(
                                        pair[:, half, :],
                                        qkT[hsl, 1, k_sl],
                                        qkT[hsl, 0, q_sl],
                                        start=True, stop=True)
                                est = work.tile([128, 2, 512], BF16,
                                                tag="est", bufs=3,
                                                name="est")
                                nc.scalar.activation(
                                    out=est.rearrange("p a t -> p (a t)"),
                                    in_=pair.rearrange("p a t -> p (a t)"),
                                    func=AFT.Exp, scale=inv_sqrt_hd)
                                m2 = pp - 2 * ql
                                if m2 >= 0:
                                    with nc.allow_low_precision(
                                            reason="bf16 mask"):
                                        nc.vector.tensor_mul(
                                            est.rearrange("p a t -> p (a t)"),
                                            est.rearrange("p a t -> p (a t)"),
                                            masks[:, m2, :])
                                ests[hh] = est

                            def av(hh, ql, pp, npair):
                                if pp == 0:
                                    psy[hh] = psum.tile([65, 512], F32,
                                                        tag="av", bufs=2,
                                                        name="ps_y")
                                for half in range(2):
                                    k = 2 * pp + half
                                    nc.tensor.matmul(
                                        psy[hh][0:65, :],
                                        vones[:, hh, (T * b) // 128 + k, :],
                                        ests[hh][:, half, :],
                                        start=(pp == 0 and half == 0),
                                        stop=(pp == npair - 1 and half == 1))

                            def finish_ql(hh, ql):
                                srow = rows_pool.tile([1, 512], BF16,
                                                      tag="sr", bufs=2,
                                                      name="srow")
                                with nc.allow_low_precision(reason="bf16"):
                                    nc.vector.reciprocal(
                                        srow[0:1, :], psy[hh][64:65, :])
                                rb = bcast_row(srow[0:1, :], 64, BF16,
                                               tag="rb")
                                with nc.allow_low_precision(
                                        reason="bf16 y"):
                                    nc.vector.tensor_mul(
                                        yT[64 * hh:64 * hh + 64, b,
                                           512 * ql:512 * (ql + 1)],
                                        psy[hh][0:64, :], rb[0:64, :])

                            for (ql, pp) in pair_list:
                                npair = 2 * ql + 2
                                qk_exp(0, ql, pp)
                                qk_exp(1, ql, pp)
                                av(0, ql, pp, npair)
                                av(1, ql, pp, npair)
                                if pp == npair - 1:
                                    finish_ql(0, ql)
                                    finish_ql(1, ql)
                            if phases <= 3:
                                continue
                            # ---- per-batch AllToAll, all on gpsimd ----
                            nc.gpsimd.dma_start(
                                out=a2a_in[b].rearrange("j p t -> p j t"),
                                in_=yT[:, b, :].rearrange(
                                    "p (j t) -> p j t", j=NCORES))
                            nc.gpsimd.collective_compute(
                                "AllToAll", ALU.bypass,
                                replica_groups=[list(range(NCORES))],
                                ins=[a2a_in[b][:]], outs=[a2a_out[b][:]],
                            )
                            nc.gpsimd.dma_start(
                                out=yfull[:, :, 256 * b:256 * (b + 1)],
                                in_=a2a_out[b].rearrange("j p t -> p j t"))
                if phases <= 4:
                    return

                # ==== Phase 4: proj+resid then LN2, per batch half so the
                # ==== b0 half runs while A2A(b1) flies
                with ExitStack() as pp_scope:
                    psp = pp_scope.enter_context(
                        tc.tile_pool(name=f"psp{rep}", bufs=1,
                                     space="PSUM"))
                    for bh in range(2):
                        tsl = slice(256 * bh, 256 * (bh + 1))
                        for co in range(NC_BLK):
                            ps = psp.tile([128, 512], F32, tag="pf",
                                          bufs=4, name="ps_pj")
                            for ci in range(NC_BLK):
                                nc.tensor.matmul(
                                    ps[:, 0:256],
                                    wp_all[:, ci, co, :],
                                    yfull[:, ci, tsl],
                                    start=(ci == 0),
                                    stop=(ci == NC_BLK - 1))
                            with nc.allow_low_precision(
                                    reason="bf16 resid"):
                                nc.vector.scalar_tensor_tensor(
                                    out=residT[:, co, tsl],
                                    in0=ps[:, 0:256],
                                    scalar=sb_bproj[:, co:co + 1],
                                    in1=xct[:, co, tsl],
                                    op0=ALU.add, op1=ALU.add)
                        if phases <= 5:
                            continue
                        # ---- LN2 for this half ----
                        sq2 = work.tile([128, NC_BLK, 256], BF16,
                                        tag="sq2", bufs=2, name="sq2")
                        with nc.allow_low_precision(reason="bf16 sq"):
                            for c in range(NC_BLK):
                                nc.vector.tensor_mul(
                                    sq2[:, c, :], residT[:, c, tsl],
                                    residT[:, c, tsl])
                        rstd2, nmu2, _ = ln_stats_rows(
                            [residT[:, c, tsl] for c in range(NC_BLK)],
                            [sq2[:, c, :] for c in range(NC_BLK)],
                            width=256)
                        nmr2 = rows_pool.tile([1, 256], BF16, tag="sr",
                                              bufs=2, name="nmr2")
                        with nc.allow_low_precision(reason="bf16"):
                            nc.vector.tensor_mul(nmr2, nmu2, rstd2)
                        rstd2_b = bcast_row(rstd2[0:1, :], 128, BF16,
                                            width=256)
                        nmr2_b = bcast_row(nmr2[0:1, :], 128, BF16,
                                           tag="bc2", width=256)
                        with nc.allow_low_precision(reason="bf16 ln2"):
                            for c in range(NC_BLK):
                                t5 = work.tile([128, 256], BF16,
                                               tag="wk", name="t5")
                                nc.vector.tensor_mul(
                                    t5, residT[:, c, tsl], rstd2_b)
                                nc.vector.tensor_add(
                                    yfull[:, c, tsl], t5, nmr2_b)
                    if phases <= 6:
                        return

                    # ---- Phase 5: MLP fc + gelu, full 512-token width ----
                    for fg in range(NF_BLK // 4):
                        if fg + 2 < NF_BLK // 4:
                            wf_load(fg + 2)
                        wf_t = wf_tiles[fg]
                        for fo in range(4):
                            f = 4 * fg + fo
                            ps = psp.tile([128, 512], F32,
                                          tag="pf", bufs=4,
                                          name="ps_fc")
                            for c in range(NC_BLK):
                                nc.tensor.matmul(
                                    ps,
                                    wf_t[:, fo, c, :],
                                    yfull[:, c, :],
                                    start=(c == 0),
                                    stop=(c == NC_BLK - 1))
                            nc.scalar.activation(
                                out=hT[:, f, :], in_=ps,
                                func=AFT.Gelu,
                                bias=sb_bfc[:, f:f + 1])
                    if phases <= 7:
                        return

                    # ---- Phase 6: MLP proj + residual + out ----
                    for co in range(NC_BLK):
                        if co + 2 < NC_BLK:
                            wm_load(co + 2)
                        wm = wm_tiles[co]
                        ps = psp.tile([128, 512], F32, tag="pf",
                                      bufs=4, name="ps_mp")
                        for f in range(NF_BLK):
                            nc.tensor.matmul(ps, wm[:, f, :],
                                             hT[:, f, :],
                                             start=(f == 0),
                                             stop=(f == NF_BLK - 1))
                        yo = work.tile([128, 512], F32, tag="yo",
                                       bufs=2, name="yo")
                        nc.vector.scalar_tensor_tensor(
                            out=yo, in0=ps,
                            scalar=sb_bmlp[:, co:co + 1],
                            in1=residT[:, co, :],
                            op0=ALU.add, op1=ALU.add)
                        nc.sync.dma_start(out=yout[:, co, :], in_=yo)

        for _rep in range(repeat):
            emit_body(_rep)

    nc.compile()
    return nc


_NC_CACHE = {}


def _get_program(repeat=1):
    if repeat not in _NC_CACHE:
        _NC_CACHE[repeat] = build_program(repeat)
    return _NC_CACHE[repeat]


def prepare_inputs(x, ln1_g, ln1_b, w_attn, b_attn, w_attn_proj, b_attn_proj,
                   ln2_g, ln2_b, w_fc, b_fc, w_mlp_proj, b_mlp_proj):
    """Host-side fold/slice/block. Returns in_maps (list of 8 dicts)."""
    f = np.float32
    bf = NPBF16
    x = np.asarray(x, f)
    # fold LN1 gain into w_attn, LN2 gain into w_fc (exact: reference
    # applies g/b after normalization; W'.T @ (g*xn + b) = (g*W)'.T @ xn
    # + (b @ W))
    w_attn_e = (np.asarray(ln1_g, f)[:, None] * np.asarray(w_attn, f))
    b_attn_e = np.asarray(ln1_b, f) @ np.asarray(w_attn, f) + \
        np.asarray(b_attn, f)
    w_fc_e = (np.asarray(ln2_g, f)[:, None] * np.asarray(w_fc, f))
    b_fc_e = np.asarray(ln2_b, f) @ np.asarray(w_fc, f) + np.asarray(b_fc, f)
    colsum = w_attn_e.sum(axis=0, dtype=np.float64).astype(f)

    xT = np.concatenate([x[0].T, x[1].T], axis=1)          # [C, 4096]
    xT_blk = np.ascontiguousarray(
        xT.reshape(NC_BLK, 128, TT).transpose(1, 0, 2)).astype(bf)

    # wproj: [C, C] -> [p(ci-row), ci, co, k]
    wp = np.ascontiguousarray(
        np.asarray(w_attn_proj, f).reshape(NC_BLK, 128, NC_BLK, 128)
        .transpose(1, 0, 2, 3)).astype(bf)
    # wfc: [C, 4C] -> [p(c-row), fg, fo, c, k] (1MB per-fg DMA slices)
    wf = np.ascontiguousarray(
        w_fc_e.reshape(NC_BLK, 128, NF_BLK // 4, 4, 128)
        .transpose(1, 2, 3, 0, 4)).astype(bf)
    # wmlp: [4C, C] -> [co, p(f-row), f, k]
    wm = np.ascontiguousarray(
        np.asarray(w_mlp_proj, f).reshape(NF_BLK, 128, NC_BLK, 128)
        .transpose(2, 1, 0, 3)).astype(bf)

    def rows_t(v, nb):
        return np.ascontiguousarray(np.asarray(v, f).reshape(nb, 128).T)

    bproj_t = rows_t(b_attn_proj, NC_BLK)
    bfc_t = rows_t(b_fc_e, NF_BLK)
    bmlp_t = rows_t(b_mlp_proj, NC_BLK)

    ones_arr = np.ones((128, 512), bf)
    ident_arr = np.eye(128).astype(bf)
    eps_arr = np.full((128, 1), EPS, f)
    # causal mask m: [p, c] valid (1.0) iff p + 128*m <= c
    p_idx = np.arange(128)[:, None]
    c_idx = np.arange(512)[None, :]
    cmask_arr = np.ascontiguousarray(np.stack(
        [(p_idx + 128 * m <= c_idx).astype(f) for m in range(4)],
        axis=1)).astype(bf)

    in_maps = []
    for i in range(NCORES):
        qcols = slice(128 * i, 128 * (i + 1))
        kcols = slice(C + 128 * i, C + 128 * (i + 1))
        vcols = slice(2 * C + 128 * i, 2 * C + 128 * (i + 1))
        wq = np.empty((128, NC_BLK, 3, 128), f)
        for c in range(NC_BLK):
            rsl = slice(128 * c, 128 * (c + 1))
            wq[:, c, 0, :] = w_attn_e[rsl, qcols]
            wq[:, c, 1, :] = w_attn_e[rsl, kcols]
            wq[:, c, 2, :] = w_attn_e[rsl, vcols]
        cb = np.empty((1, 6, 128), f)
        for j, sl in enumerate((qcols, kcols, vcols)):
            cb[0, j, :] = colsum[sl]
            cb[0, 3 + j, :] = b_attn_e[sl]
        # per-core residual chunk: b0[256i:256(i+1)] ++ b1[256i:256(i+1)]
        xc = np.concatenate(
            [xT_blk[:, :, 256 * i:256 * (i + 1)],
             xT_blk[:, :, 2048 + 256 * i:2048 + 256 * (i + 1)]], axis=2)
        in_maps.append({
            "ones_in": ones_arr,
            "cmask": cmask_arr,
            "identin": ident_arr,
            "epsin": eps_arr,
            "xT": xT_blk,
            "xc_in": np.ascontiguousarray(xc),
            "wqkv": wq.astype(bf),
            "cbqkv": cb.astype(bf),
            "wproj": wp,
            "bproj": bproj_t,
            "wfc": wf,
            "bfc": bfc_t,
            "wmlp": wm,
            "bmlp": bmlp_t,
        })
    return in_maps


def assemble_output(results):
    out = np.empty((B, T, C), np.float32)
    for i in range(NCORES):
        yo = results[i]["yout"]                      # [128, 8, 512]
        y = yo.transpose(1, 0, 2).reshape(C, CHUNK)  # [feature, 512]
        out[0, 256 * i:256 * (i + 1), :] = y[:, 0:256].T
        out[1, 256 * i:256 * (i + 1), :] = y[:, 256:512].T
    return out


def kernel(**inputs):
    nc = _get_program()
    in_maps = prepare_inputs(**inputs)
    res = run_bass_kernel_spmd(nc, in_maps, list(range(NCORES)))
    return assemble_output(res.results)


if __name__ == "__main__":
    import reference
    inputs = {k: np.asarray(v) for k, v in reference.setup_inputs().items()}
    expected = np.asarray(reference.reference(**inputs))
    actual = kernel(**inputs)
    err = np.abs(actual - expected).max() / (np.abs(expected).max() + 1e-30)
    print("Relative error:", err)


# revision 15
# speedup vs baseline: 1.1262x; 1.0393x over previous
"""Trainium2 Bass kernel for a dense transformer block (B=2, T=2048, C=1024, H=16).

v4 over v3 (all scheduling, no math changes):
  - Attention restructured as TWO interleaved head-streams per batch so
    each pair's softmax-exp (ScalarE) overlaps the other stream's QK/AV
    matmuls (v3 serialized QK->exp->AV per pair: ~2.7us/pair).
  - A2A input DMA + collective + consume DMA all on the gpsimd queue
    (v3 had the consume on the ACT queue, stalling batch-1 exps behind
    the batch-0 collective).
  - wfc/wmlp streamed on the SP HWDGE queue with all dma_starts emitted
    right after P1, so 3+3 MB prefetch during attention while HBM is
    otherwise idle (v3 issued wmlp late on the gpsimd SWDGE queue).
  - fc is fg-outer, full 512-token width: wfc read once (v3 streamed it
    per batch-half = 2x traffic) and N=512 matmuls/gelu.

Sharding: 8-way tensor parallel over heads for QKV+attention (each core
owns 2 heads over all 4096 tokens); a per-batch AllToAll switches to token
parallelism: core i owns tokens b0[256i:+256] + b1[256i:+256] for
attn-proj, LN2 and the MLP.
"""

import math
import sys
from contextlib import ExitStack

import numpy as np

for _p in ("/opt/trn_rl_repo",):
    if _p not in sys.path:
        sys.path.insert(0, _p)

import concourse.bacc as bacc
import concourse.mybir as mybir
import concourse.tile as tile
from concourse.bass_utils import run_bass_kernel_spmd

F32 = mybir.dt.float32
BF16 = mybir.dt.bfloat16
F8E3 = mybir.dt.float8e3
NPF8E3 = mybir.dt.np(mybir.dt.float8e3)
WSCALE = 64.0
NPBF16 = mybir.dt.np(mybir.dt.bfloat16)

B, T, C = 2, 2048, 1024
H, HD = 16, 64
TT = B * T              # 4096 flat tokens (b0: 0..2047, b1: 2048..4095)
NCORES = 8
CHUNK = 512             # tokens per core for the MLP part (256 per batch)
NC_BLK = C // 128       # 8 feature blocks
NF_BLK = 4 * C // 128   # 32 mlp-hidden blocks
EPS = 1e-5


def build_program(repeat=1, phases=99):
    nc = bacc.Bacc("TRN2", target_bir_lowering=False, debug=False,
                   num_devices=NCORES)

    # ---- I/O (big tensors bf16, per-partition-contiguous layouts) ----
    xT = nc.dram_tensor("xT", [128, NC_BLK, TT], BF16, kind="ExternalInput")
    xc_in = nc.dram_tensor("xc_in", [128, NC_BLK, CHUNK], BF16,
                           kind="ExternalInput")
    wqkv = nc.dram_tensor("wqkv", [128, NC_BLK, 3, 128], BF16,
                          kind="ExternalInput")
    cbqkv = nc.dram_tensor("cbqkv", [1, 6, 128], BF16, kind="ExternalInput")
    wproj = nc.dram_tensor("wproj", [128, NC_BLK, NC_BLK, 128], BF16,
                           kind="ExternalInput")
    bproj = nc.dram_tensor("bproj", [128, NC_BLK], F32, kind="ExternalInput")
    wfc = nc.dram_tensor("wfc", [128, NF_BLK // 4, 4, NC_BLK, 128], F8E3,
                         kind="ExternalInput")
    bfc = nc.dram_tensor("bfc", [128, NF_BLK], F32, kind="ExternalInput")
    wmlp = nc.dram_tensor("wmlp", [NC_BLK, 128, NF_BLK, 128], F8E3,
                          kind="ExternalInput")
    bmlp = nc.dram_tensor("bmlp", [128, NC_BLK], F32, kind="ExternalInput")
    ones_in = nc.dram_tensor("ones_in", [128, 512], BF16,
                             kind="ExternalInput")
    cmask = nc.dram_tensor("cmask", [128, 4, 512], BF16, kind="ExternalInput")
    identin = nc.dram_tensor("identin", [128, 128], BF16,
                             kind="ExternalInput")
    epsin = nc.dram_tensor("epsin", [128, 1], F32, kind="ExternalInput")
    yout = nc.dram_tensor("yout", [128, NC_BLK, CHUNK], F32,
                          kind="ExternalOutput")

    AFT = mybir.ActivationFunctionType
    ALU = mybir.AluOpType

    with tile.TileContext(nc) as tc, ExitStack() as top:
        psum = top.enter_context(tc.tile_pool(name="psum", bufs=1,
                                              space="PSUM"))
        consts = top.enter_context(tc.tile_pool(name="consts", bufs=1))
        wres = top.enter_context(tc.tile_pool(name="wres", bufs=1))
        rows_pool = top.enter_context(tc.tile_pool(name="rows", bufs=4))
        bcast_pool = top.enter_context(tc.tile_pool(name="bcast", bufs=2))
        work = top.enter_context(tc.tile_pool(name="work", bufs=4))
        dram = top.enter_context(tc.tile_pool(name="dram", bufs=1,
                                              space="DRAM"))

        # ---- constants ----
        ident = consts.tile([128, 128], BF16)
        nc.sync.dma_start(out=ident, in_=identin[:, :])
        ones_bf = consts.tile([128, 512], BF16)
        nc.sync.dma_start(out=ones_bf, in_=ones_in[:, :])
        ones_col = ones_bf[:, 0:1]
        eps_col = consts.tile([128, 1], F32)
        nc.sync.dma_start(out=eps_col, in_=epsin[:, :])
        masks = consts.tile([128, 2, 1024], BF16)  # two band pair-masks
        nc.sync.dma_start(out=masks,
                          in_=cmask.ap().rearrange("p (a b) t -> p a (b t)",
                                                   a=2))
        sb_cbq = consts.tile([1, 6, 128], BF16)
        nc.sync.dma_start(out=sb_cbq, in_=cbqkv[:, :, :])
        sb_bproj = consts.tile([128, NC_BLK], F32)
        nc.sync.dma_start(out=sb_bproj, in_=bproj[:, :])
        sb_bfc = consts.tile([128, NF_BLK], F32)
        nc.sync.dma_start(out=sb_bfc, in_=bfc[:, :])
        sb_bmlp = consts.tile([128, NC_BLK], F32)
        nc.sync.dma_start(out=sb_bmlp, in_=bmlp[:, :])

        # ---- resident weights (loaded once per program) ----
        wq_all = wres.tile([128, NC_BLK, 3, 128], BF16, name="wq_all")
        nc.sync.dma_start(out=wq_all, in_=wqkv[:, :, :, :])
        wp_all = wres.tile([128, NC_BLK, NC_BLK, 128], BF16, name="wp_all")
        nc.sync.dma_start(out=wp_all, in_=wproj[:, :, :, :])

        def bcast_row(row_ap, nparts, dtype, tag="bc", width=512):
            """Broadcast a [1, width] SBUF row to [nparts, width] via a K=1
            PE outer product with a ones row, evacuated to SBUF by DVE."""
            ps = psum.tile([128, 512], F32, tag="mm", bufs=2, name="ps_bc")
            nc.tensor.matmul(ps[0:nparts, 0:width], ones_bf[0:1, 0:nparts],
                             row_ap, start=True, stop=True)
            out = bcast_pool.tile([nparts, width], dtype, tag=tag,
                                  name="bc_row")
            with nc.allow_low_precision(reason="broadcast copy"):
                nc.vector.tensor_copy(out, ps[0:nparts, 0:width])
            return out

        def finish_stats(ps_s, ps_q, width=512):
            """Turn Σx (ps_s) and Σx² (ps_q) rows into bf16 stat rows:
            rstd [1,w] and nmsd [2,w] (row0 = -mu, row1 = sd)."""
            mu = rows_pool.tile([1, width], F32, tag="r")
            nc.vector.tensor_scalar_mul(mu, ps_s[0:1, 0:width], 1.0 / C)
            ex2 = rows_pool.tile([1, width], F32, tag="r")
            nc.vector.tensor_scalar_mul(ex2, ps_q[0:1, 0:width], 1.0 / C)
            var = rows_pool.tile([1, width], F32, tag="r")
            musq = rows_pool.tile([1, width], F32, tag="r")
            nc.vector.tensor_mul(musq, mu, mu)
            nc.vector.tensor_sub(var, ex2, musq)
            lnv = rows_pool.tile([1, width], F32, tag="r")
            nc.scalar.activation(out=lnv, in_=var, func=AFT.Ln,
                                 bias=eps_col[0:1, 0:1])
            rstd = rows_pool.tile([1, width], BF16, tag="rb", bufs=2)
            with nc.allow_low_precision(reason="bf16 rows"):
                # rstd = exp(-0.5*ln(var+eps)) = rsqrt(var+eps); Ln/Exp
                # share one ACT table set with the attention exps.
                nc.scalar.activation(out=rstd, in_=lnv, func=AFT.Exp,
                                     scale=-0.5)
            sdb = rows_pool.tile([1, width], BF16, tag="rb", bufs=2)
            nmu = rows_pool.tile([1, width], BF16, tag="rb", bufs=2)
            with nc.allow_low_precision(reason="bf16 rows"):
                nc.vector.reciprocal(sdb, rstd)
                nc.vector.tensor_scalar_mul(nmu, mu, -1.0)
            return rstd, nmu, sdb

        def ln_stats_rows(srcs, sqs, width=512):
            """srcs/sqs: NC_BLK [128, width] bf16 APs (feature blocks of one
            token chunk and their elementwise squares)."""
            ps_s = psum.tile([65, 512], F32, tag="av", bufs=2, name="ps_s")
            ps_q = psum.tile([65, 512], F32, tag="av", bufs=2, name="ps_q")
            for c in range(NC_BLK):
                nc.tensor.matmul(ps_s[0:1, 0:width], ones_col, srcs[c],
                                 start=(c == 0), stop=(c == NC_BLK - 1))
            for c in range(NC_BLK):
                nc.tensor.matmul(ps_q[0:1, 0:width], ones_col, sqs[c],
                                 start=(c == 0), stop=(c == NC_BLK - 1))
            return finish_stats(ps_s, ps_q, width)

        def emit_body(rep):
            with ExitStack() as body_scope:
                body = body_scope.enter_context(
                    tc.tile_pool(name=f"body{rep}", bufs=1))
                wstream = body_scope.enter_context(
                    tc.tile_pool(name=f"wstr{rep}", bufs=1))
                xct = body.tile([128, NC_BLK, CHUNK], BF16, name="xct")
                residT = body.tile([128, NC_BLK, CHUNK], BF16, name="residT")
                hT = body.tile([128, NF_BLK, CHUNK], BF16, name="hT")
                yfull = body.tile([128, NC_BLK, CHUNK], BF16, name="yfull")

                with ExitStack() as attn_scope:
                    attn_pool = attn_scope.enter_context(
                        tc.tile_pool(name=f"attn{rep}", bufs=1))
                    qkT = attn_pool.tile([128, 2, TT], BF16, name="qkT")
                    vones = attn_pool.tile([128, 2, TT // 128, 65], BF16,
                                           name="vones")
                    yT = attn_pool.tile([128, B, T], BF16, name="yT")
                    nc.vector.tensor_copy(
                        vones[:, :, :, 64:65].rearrange(
                            "p a b k -> p (a b k)"),
                        ones_bf[:, 0:64])

                    # ========== Phase 1: LN1 stats + QKV ==========
                    with ExitStack() as p1_scope:
                        xc_pool = p1_scope.enter_context(
                            tc.tile_pool(name=f"xcp{rep}", bufs=2))
                        psq = p1_scope.enter_context(
                            tc.tile_pool(name=f"psq{rep}", bufs=1,
                                         space="PSUM"))
                        for qi in range(8):
                            csl = slice(512 * qi, 512 * (qi + 1))
                            xTc = xc_pool.tile([128, NC_BLK, 512], BF16,
                                               tag="xTc", name="xTc")
                            nc.sync.dma_start(out=xTc, in_=xT[:, :, csl])
                            srcs = [xTc[:, c, :] for c in range(NC_BLK)]
                            ps_s = psum.tile([65, 512], F32, tag="av",
                                             bufs=2, name="ps_s")
                            ps_q = psum.tile([65, 512], F32, tag="av",
                                             bufs=2, name="ps_q")
                            for c in range(NC_BLK):
                                nc.tensor.matmul(ps_s[0:1, :], ones_col,
                                                 srcs[c], start=(c == 0),
                                                 stop=(c == NC_BLK - 1))
                            for cc in range(4):
                                sqt = xc_pool.tile([128, 2, 512], BF16,
                                                   tag="sq", bufs=2,
                                                   name="sq")
                                with nc.allow_low_precision(reason="sq"):
                                    for k in range(2):
                                        nc.vector.tensor_mul(
                                            sqt[:, k, :], srcs[2 * cc + k],
                                            srcs[2 * cc + k])
                                for k in range(2):
                                    nc.tensor.matmul(
                                        ps_q[0:1, :], ones_col,
                                        sqt[:, k, :],
                                        start=(cc == 0 and k == 0),
                                        stop=(cc == 3 and k == 1))
                            rstd, nmu, sdb = finish_stats(ps_s, ps_q)
                            rstd_b = bcast_row(rstd[0:1, :], 128, BF16)
                            ps_j = []
                            for j in range(3):
                                ps = psq.tile([128, 512], F32, tag="q5",
                                              bufs=4, name="ps_qkv")
                                for c in range(NC_BLK):
                                    nc.tensor.matmul(ps, wq_all[:, c, j, :],
                                                     srcs[c],
                                                     start=(c == 0),
                                                     stop=False)
                                nc.tensor.matmul(ps, sb_cbq[:, j, :],
                                                 nmu[:, :],
                                                 start=False, stop=False)
                                nc.tensor.matmul(ps, sb_cbq[:, 3 + j, :],
                                                 sdb[:, :],
                                                 start=False, stop=True)
                                ps_j.append(ps)
                            for j in range(3):
                                with nc.allow_low_precision(reason="bf16"):
                                    if j < 2:
                                        nc.vector.tensor_mul(
                                            qkT[:, j, csl], ps_j[j], rstd_b)
                                    else:
                                        vch = work.tile([128, 512], BF16,
                                                        tag="vch", bufs=2,
                                                        name="vch")
                                        nc.vector.tensor_mul(vch, ps_j[j],
                                                             rstd_b)
                                        for kb in range(4):
                                            ps_t = psum.tile(
                                                [128, 128], BF16,
                                                tag="mm", bufs=2,
                                                name="ps_tr")
                                            nc.tensor.transpose(
                                                ps_t,
                                                vch[:, 128 * kb:
                                                    128 * (kb + 1)],
                                                ident)
                                            gb = 4 * qi + kb
                                            for hh in range(2):
                                                nc.vector.tensor_copy(
                                                    vones[:, hh, gb, 0:64],
                                                    ps_t[:, 64 * hh:
                                                         64 * hh + 64])

                    # residual chunk for this core (lands during attention)
                    nc.sync.dma_start(out=xct, in_=xc_in[:, :, :])

                    # weight streams on the SP HWDGE queue. The first 3+3
                    # loads are emitted here so they fire during attention
                    # while HBM is idle; the rest are emitted inside the
                    # fc/mlp loops with bufs-deep lookahead (emission order
                    # = SP queue order, so no head-of-line blocking).
                    wf_tiles = {}
                    wm_tiles = {}

                    def wf_load(fg):
                        t = wstream.tile([128, 4, NC_BLK, 128], F8E3,
                                         tag="wf", bufs=3, name=f"wf{fg}")
                        nc.sync.dma_start(out=t, in_=wfc[:, fg, :, :, :])
                        wf_tiles[fg] = t

                    def wm_load(co):
                        t = wstream.tile([128, NF_BLK, 128], F8E3,
                                         tag="wm", bufs=3, name=f"wm{co}")
                        nc.sync.dma_start(out=t, in_=wmlp[co, :, :, :])
                        wm_tiles[co] = t

                    for fg in range(3):
                        wf_load(fg)
                    for co in range(3):
                        wm_load(co)
                    if phases <= 1:
                        return

                    # ====== Phase 2: causal attention, two interleaved
                    # ====== head-streams per batch ======
                    a2a_in = [dram.tile([NCORES, 128, 256], BF16,
                                        name=f"a2a_in{b}") for b in range(B)]
                    a2a_out = [dram.tile([NCORES, 128, 256], BF16,
                                         name=f"a2a_out{b}")
                               for b in range(B)]
                    inv_sqrt_hd = 1.0 / math.sqrt(HD)
                    pair_list = [(ql, pp) for ql in range(4)
                                 for pp in range(2 * ql + 2)]
                    with ExitStack() as pa_scope:
                        psa = pa_scope.enter_context(
                            tc.tile_pool(name=f"psa{rep}", bufs=1,
                                         space="PSUM"))
                        for b in range(B):
                            psy = [None, None]
                            ests = [None, None]

                            def qk_exp(hh, ql, pp):
                                hsl = slice(64 * hh, 64 * hh + 64)
                                q_sl = slice(T * b + 512 * ql,
                                             T * b + 512 * (ql + 1))
                                pair = psa.tile([128, 2, 512], F32,
                                                tag="qk", bufs=2,
                                                name="ps_qk")
                                for half in range(2):
                                    k = 2 * pp + half
                                    k_sl = slice(T * b + 128 * k,
                                                 T * b + 128 * (k + 1))
                                    nc.tensor.matmul(
                                        pair[:, half, :],
                                        qkT[hsl, 1, k_sl],
                                        qkT[hsl, 0, q_sl],
                                        start=True, stop=True)
                                est = work.tile([128, 2, 512], BF16,
                                                tag="est", bufs=3,
                                                name="est")
                                nc.scalar.activation(
                                    out=est.rearrange("p a t -> p (a t)"),
                                    in_=pair.rearrange("p a t -> p (a t)"),
                                    func=AFT.Exp, scale=inv_sqrt_hd)
                                m2 = pp - 2 * ql
                                if m2 >= 0:
                                    with nc.allow_low_precision(
                                            reason="bf16 mask"):
                                        nc.vector.tensor_mul(
                                            est.rearrange("p a t -> p (a t)"),
                                            est.rearrange("p a t -> p (a t)"),
                                            masks[:, m2, :])
                                ests[hh] = est

                            def av(hh, ql, pp, npair):
                                if pp == 0:
                                    psy[hh] = psum.tile([65, 512], F32,
                                                        tag="av", bufs=2,
                                                        name="ps_y")
                                for half in range(2):
                                    k = 2 * pp + half
                                    nc.tensor.matmul(
                                        psy[hh][0:65, :],
                                        vones[:, hh, (T * b) // 128 + k, :],
                                        ests[hh][:, half, :],
                                        start=(pp == 0 and half == 0),
                                        stop=(pp == npair - 1 and half == 1))

                            def finish_ql(hh, ql):
                                srow = rows_pool.tile([1, 512], BF16,
                                                      tag="sr", bufs=2,
                                                      name="srow")
                                with nc.allow_low_precision(reason="bf16"):
                                    nc.vector.reciprocal(
                                        srow[0:1, :], psy[hh][64:65, :])
                                rb = bcast_row(srow[0:1, :], 64, BF16,
                                               tag="rb")
                                with nc.allow_low_precision(
                                        reason="bf16 y"):
                                    nc.vector.tensor_mul(
                                        yT[64 * hh:64 * hh + 64, b,
                                           512 * ql:512 * (ql + 1)],
                                        psy[hh][0:64, :], rb[0:64, :])

                            # finish_ql is deferred one slot so its DVE->PE
                            # broadcast chain hides under the next slot's
                            # QK/exp instead of stalling the PE queue.
                            pending = None
                            for (ql, pp) in pair_list:
                                npair = 2 * ql + 2
                                qk_exp(0, ql, pp)
                                qk_exp(1, ql, pp)
                                if pending is not None:
                                    finish_ql(0, pending)
                                    finish_ql(1, pending)
                                    pending = None
                                av(0, ql, pp, npair)
                                av(1, ql, pp, npair)
                                if pp == npair - 1:
                                    pending = ql
                            finish_ql(0, pending)
                            finish_ql(1, pending)
                            if phases <= 3:
                                continue
                            # ---- per-batch AllToAll, all on gpsimd ----
                            nc.gpsimd.dma_start(
                                out=a2a_in[b].rearrange("j p t -> p j t"),
                                in_=yT[:, b, :].rearrange(
                                    "p (j t) -> p j t", j=NCORES))
                            nc.gpsimd.collective_compute(
                                "AllToAll", ALU.bypass,
                                replica_groups=[list(range(NCORES))],
                                ins=[a2a_in[b][:]], outs=[a2a_out[b][:]],
                            )
                            nc.gpsimd.dma_start(
                                out=yfull[:, :, 256 * b:256 * (b + 1)],
                                in_=a2a_out[b].rearrange("j p t -> p j t"))
                if phases <= 4:
                    return

                # ==== Phase 4: proj+resid then LN2, per batch half so the
                # ==== b0 half runs while A2A(b1) flies
                with ExitStack() as pp_scope:
                    psp = pp_scope.enter_context(
                        tc.tile_pool(name=f"psp{rep}", bufs=1,
                                     space="PSUM"))
                    for bh in range(2):
                        tsl = slice(256 * bh, 256 * (bh + 1))
                        for co in range(NC_BLK):
                            ps = psp.tile([128, 512], F32, tag="pf",
                                          bufs=4, name="ps_pj")
                            for ci in range(NC_BLK):
                                nc.tensor.matmul(
                                    ps[:, 0:256],
                                    wp_all[:, ci, co, :],
                                    yfull[:, ci, tsl],
                                    start=(ci == 0),
                                    stop=(ci == NC_BLK - 1))
                            with nc.allow_low_precision(
                                    reason="bf16 resid"):
                                nc.vector.scalar_tensor_tensor(
                                    out=residT[:, co, tsl],
                                    in0=ps[:, 0:256],
                                    scalar=sb_bproj[:, co:co + 1],
                                    in1=xct[:, co, tsl],
                                    op0=ALU.add, op1=ALU.add)
                        if phases <= 5:
                            continue
                        # ---- LN2 for this half ----
                        sq2 = work.tile([128, NC_BLK, 256], BF16,
                                        tag="sq2", bufs=2, name="sq2")
                        with nc.allow_low_precision(reason="bf16 sq"):
                            for c in range(NC_BLK):
                                nc.vector.tensor_mul(
                                    sq2[:, c, :], residT[:, c, tsl],
                                    residT[:, c, tsl])
                        rstd2, nmu2, _ = ln_stats_rows(
                            [residT[:, c, tsl] for c in range(NC_BLK)],
                            [sq2[:, c, :] for c in range(NC_BLK)],
                            width=256)
                        nmr2 = rows_pool.tile([1, 256], BF16, tag="sr",
                                              bufs=2, name="nmr2")
                        with nc.allow_low_precision(reason="bf16"):
                            nc.vector.tensor_mul(nmr2, nmu2, rstd2)
                        rstd2_b = bcast_row(rstd2[0:1, :], 128, BF16,
                                            width=256)
                        nmr2_b = bcast_row(nmr2[0:1, :], 128, BF16,
                                           tag="bc2", width=256)
                        with nc.allow_low_precision(reason="bf16 ln2"):
                            for c in range(NC_BLK):
                                t5 = work.tile([128, 256], BF16,
                                               tag="wk", name="t5")
                                nc.vector.tensor_mul(
                                    t5, residT[:, c, tsl], rstd2_b)
                                nc.vector.tensor_add(
                                    yfull[:, c, tsl], t5, nmr2_b)
                    if phases <= 6:
                        return

                    # ---- Phase 5: MLP fc + gelu, full 512-token width ----
                    for fg in range(NF_BLK // 4):
                        if fg + 3 < NF_BLK // 4:
                            wf_load(fg + 3)
                        wf_t = wf_tiles[fg]
                        for fo in range(4):
                            f = 4 * fg + fo
                            ps = psp.tile([128, 512], F32,
                                          tag="pf", bufs=4,
                                          name="ps_fc")
                            for c in range(NC_BLK):
                                nc.tensor.matmul(
                                    ps,
                                    wf_t[:, fo, c, :],
                                    yfull[:, c, :],
                                    start=(c == 0),
                                    stop=(c == NC_BLK - 1))
                            nc.scalar.activation(
                                out=hT[:, f, :], in_=ps,
                                func=AFT.Gelu, scale=1.0 / WSCALE,
                                bias=sb_bfc[:, f:f + 1])
                    if phases <= 7:
                        return

                    # ---- Phase 6: MLP proj + residual + out ----
                    for co in range(NC_BLK):
                        if co + 3 < NC_BLK:
                            wm_load(co + 3)
                        wm = wm_tiles[co]
                        ps = psp.tile([128, 512], F32, tag="pf",
                                      bufs=4, name="ps_mp")
                        for f in range(NF_BLK):
                            nc.tensor.matmul(ps, wm[:, f, :],
                                             hT[:, f, :],
                                             start=(f == 0),
                                             stop=(f == NF_BLK - 1))
                        yt = work.tile([128, 512], F32, tag="yt",
                                       bufs=2, name="yt")
                        nc.scalar.activation(
                            out=yt, in_=ps, func=AFT.Identity,
                            scale=1.0 / WSCALE,
                            bias=sb_bmlp[:, co:co + 1])
                        yo = work.tile([128, 512], F32, tag="yo",
                                       bufs=2, name="yo")
                        nc.vector.tensor_add(yo, yt, residT[:, co, :])
                        nc.sync.dma_start(out=yout[:, co, :], in_=yo)

        for _rep in range(repeat):
            emit_body(_rep)

    nc.compile()
    return nc


_NC_CACHE = {}


def _get_program(repeat=1):
    if repeat not in _NC_CACHE:
        _NC_CACHE[repeat] = build_program(repeat)
    return _NC_CACHE[repeat]


def prepare_inputs(x, ln1_g, ln1_b, w_attn, b_attn, w_attn_proj, b_attn_proj,
                   ln2_g, ln2_b, w_fc, b_fc, w_mlp_proj, b_mlp_proj):
    """Host-side fold/slice/block. Returns in_maps (list of 8 dicts)."""
    f = np.float32
    bf = NPBF16
    x = np.asarray(x, f)
    # fold LN1 gain into w_attn, LN2 gain into w_fc (exact: reference
    # applies g/b after normalization; W'.T @ (g*xn + b) = (g*W)'.T @ xn
    # + (b @ W))
    w_attn_e = (np.asarray(ln1_g, f)[:, None] * np.asarray(w_attn, f))
    b_attn_e = np.asarray(ln1_b, f) @ np.asarray(w_attn, f) + \
        np.asarray(b_attn, f)
    w_fc_e = (np.asarray(ln2_g, f)[:, None] * np.asarray(w_fc, f))
    b_fc_e = np.asarray(ln2_b, f) @ np.asarray(w_fc, f) + np.asarray(b_fc, f)
    colsum = w_attn_e.sum(axis=0, dtype=np.float64).astype(f)

    xT = np.concatenate([x[0].T, x[1].T], axis=1)          # [C, 4096]
    xT_blk = np.ascontiguousarray(
        xT.reshape(NC_BLK, 128, TT).transpose(1, 0, 2)).astype(bf)

    # wproj: [C, C] -> [p(ci-row), ci, co, k]
    wp = np.ascontiguousarray(
        np.asarray(w_attn_proj, f).reshape(NC_BLK, 128, NC_BLK, 128)
        .transpose(1, 0, 2, 3)).astype(bf)
    # wfc: [C, 4C] -> [p(c-row), fg, fo, c, k], e3m4 at WSCALE=64
    wf = np.ascontiguousarray(
        np.clip(w_fc_e * 64.0, -15.5, 15.5)
        .reshape(NC_BLK, 128, NF_BLK // 4, 4, 128)
        .transpose(1, 2, 3, 0, 4)).astype(NPF8E3)
    # wmlp: [4C, C] -> [co, p(f-row), f, k], e3m4 at WSCALE=64
    wm = np.ascontiguousarray(
        np.clip(np.asarray(w_mlp_proj, f) * 64.0, -15.5, 15.5)
        .reshape(NF_BLK, 128, NC_BLK, 128)
        .transpose(2, 1, 0, 3)).astype(NPF8E3)

    def rows_t(v, nb):
        return np.ascontiguousarray(np.asarray(v, f).reshape(nb, 128).T)

    bproj_t = rows_t(b_attn_proj, NC_BLK)
    bfc_t = rows_t(b_fc_e, NF_BLK)
    bmlp_t = rows_t(b_mlp_proj, NC_BLK)

    ones_arr = np.ones((128, 512), bf)
    ident_arr = np.eye(128).astype(bf)
    eps_arr = np.full((128, 1), EPS, f)
    # causal mask m: [p, c] valid (1.0) iff p + 128*m <= c
    p_idx = np.arange(128)[:, None]
    c_idx = np.arange(512)[None, :]
    cmask_arr = np.ascontiguousarray(np.stack(
        [(p_idx + 128 * m <= c_idx).astype(f) for m in range(4)],
        axis=1)).astype(bf)

    in_maps = []
    for i in range(NCORES):
        qcols = slice(128 * i, 128 * (i + 1))
        kcols = slice(C + 128 * i, C + 128 * (i + 1))
        vcols = slice(2 * C + 128 * i, 2 * C + 128 * (i + 1))
        wq = np.empty((128, NC_BLK, 3, 128), f)
        for c in range(NC_BLK):
            rsl = slice(128 * c, 128 * (c + 1))
            wq[:, c, 0, :] = w_attn_e[rsl, qcols]
            wq[:, c, 1, :] = w_attn_e[rsl, kcols]
            wq[:, c, 2, :] = w_attn_e[rsl, vcols]
        cb = np.empty((1, 6, 128), f)
        for j, sl in enumerate((qcols, kcols, vcols)):
            cb[0, j, :] = colsum[sl]
            cb[0, 3 + j, :] = b_attn_e[sl]
        # per-core residual chunk: b0[256i:256(i+1)] ++ b1[256i:256(i+1)]
        xc = np.concatenate(
            [xT_blk[:, :, 256 * i:256 * (i + 1)],
             xT_blk[:, :, 2048 + 256 * i:2048 + 256 * (i + 1)]], axis=2)
        in_maps.append({
            "ones_in": ones_arr,
            "cmask": cmask_arr,
            "identin": ident_arr,
            "epsin": eps_arr,
            "xT": xT_blk,
            "xc_in": np.ascontiguousarray(xc),
            "wqkv": wq.astype(bf),
            "cbqkv": cb.astype(bf),
            "wproj": wp,
            "bproj": bproj_t,
            "wfc": wf,
            "bfc": bfc_t,
            "wmlp": wm,
            "bmlp": bmlp_t,
        })
    return in_maps


def assemble_output(results):
    out = np.empty((B, T, C), np.float32)
    for i in range(NCORES):
        yo = results[i]["yout"]                      # [128, 8, 512]
        y = yo.transpose(1, 0, 2).reshape(C, CHUNK)  # [feature, 512]
        out[0, 256 * i:256 * (i + 1), :] = y[:, 0:256].T
        out[1, 256 * i:256 * (i + 1), :] = y[:, 256:512].T
    return out


def kernel(**inputs):
    nc = _get_program()
    in_maps = prepare_inputs(**inputs)
    res = run_bass_kernel_spmd(nc, in_maps, list(range(NCORES)))
    return assemble_output(res.results)


if __name__ == "__main__":
    import reference
    inputs = {k: np.asarray(v) for k, v in reference.setup_inputs().items()}
    expected = np.asarray(reference.reference(**inputs))
    actual = kernel(**inputs)
    err = np.abs(actual - expected).max() / (np.abs(expected).max() + 1e-30)
    print("Relative error:", err)
